# revision 1
# baseline (speedup 1.0000x reference)
"""Trainium2 Bass kernel for nn_Net_53644141527184.

Computation (per batch b):
  For each branch br in {x1, x3, x5}:
    picked[b, g, p] = x_br[b, idx[br, g, p], p]          (channel gather, p = 0..255)
    grid = picked.reshape(B, 128, 16, 16)
    crop[b, g, i, j] = grid[b, g, oh[g]+i, ow[g]+j]      (per-group 14x14 crop)
  feats = concat(crops, axis=1)                          -> [B, 384, 14, 14]
  out = einsum('bchw,oc->bohw', feats, W)                -> [B, 768, 14, 14]

Strategy: pure data parallel over 8 NeuronCores (16 batches each).
Inputs are sharded by batch; x is relaid out host-side (pure data movement)
to [k=32, (p_local 8 x b 16)=128, c=512] so that each SBUF tile has
partition = (position, batch) with 16-partition groups = one grid position.

Groups g are stable-sorted host-side by crop offset v = 2*oh+ow and assigned
to a 32-aligned padded slot layout (pad slots gather channel 0 and carry
zero W rows / zeroed feats rows, so they contribute nothing).  This makes
every crop band start at a 32-aligned partition, which the vector/scalar
engines require.

Per core pipeline:
  1. DMA x tiles [128, 4*512] f32 (2KB contiguous runs).
  2. gpsimd.ap_gather with S (~160) padded slots: per-16-partition-group
     index lists = per-position channel picks shared by the 16 batches of
     the group -> picked [(p_local, b), S slots] f32.
  3. PE transpose per 128-slot chunk -> PSUM [slot, (p_local, b)].
  4. Crop fused into the PSUM->SBUF copy: per offset band (32-aligned slot
     range), a strided DVE/ACT copy with the band's constant shift writes
     valid positions straight into the conv-K tiles
     feats[tile][slot_in_tile, (b16, q196)] bf16.  Residual slots (>128)
     of all branches pack into shared extra conv-K tiles.
  5. 1x1 conv: out[o_chunk, (2b x 196q)] = sum over conv-K tiles of
     WT_tile^T @ feats_tile (bf16 matmuls, f32 PSUM), copied to SBUF,
     DMA'd out with 784B contiguous runs.
Index arrays / W / x are preprocessed host-side into device-friendly
layouts (pure relayout; all arithmetic happens on device).
"""

import numpy as np
from contextlib import ExitStack

import concourse.bacc as bacc
import concourse.bass as bass
import concourse.tile as tile
import concourse.mybir as mybir
from concourse import bass_utils, masks

N_CORES = 8
B = 16        # batches per core
C = 512
P = 256       # grid positions (16x16)
G = 128       # groups per branch
NQ = 196      # cropped positions (14x14)
BR = 3
OC = 768
NK = 32       # position blocks of 8 (k = 2*row + col_half)

_CACHE = {}


def _band_geometry(k, dh, dw):
    """Valid p_local range and target q offset for k-block under shift (dh,dw).

    k covers grid row r = k//2, cols c0..c0+7 (c0 = 8*(k%2)).
    Returns (pl_lo, pl_hi, q_base) or None; q = q_base + (pl - pl_lo).
    """
    r = k // 2
    c0 = 8 * (k % 2)
    qr = r - dh
    if not (0 <= qr < 14):
        return None
    pl_lo = max(0, dw - c0)
    pl_hi = min(8, 14 + dw - c0)
    if pl_lo >= pl_hi:
        return None
    q_base = 14 * qr + (c0 + pl_lo - dw)
    return pl_lo, pl_hi, q_base


def _plan(offh, offw):
    """Compute the padded slot layout and conv-tile packing."""
    v = 2 * offh.astype(int) + offw.astype(int)
    perms = [np.argsort(v[br], kind="stable") for br in range(BR)]
    plan = {"perms": perms, "S": [], "bands": [], "pieces": []}
    for br in range(BR):
        cnt = np.bincount(v[br], minlength=4)
        slot = 0
        bands = []
        for vv in range(4):
            n = int(cnt[vv])
            bands.append((vv, slot, n))
            slot += ((n + 31) // 32) * 32
        S = max(slot, 128)
        if S % 4:
            S += 4 - (S % 4)
        plan["S"].append(S)
        plan["bands"].append(bands)

    # residual chunks (slots >= 128) pack greedily into extra tiles
    resid_assign = {}
    bins = []
    for br in range(BR):
        sz = plan["S"][br] - 128
        sz = ((sz + 31) // 32) * 32
        if sz <= 0:
            continue
        placed = False
        for i in range(len(bins)):
            if bins[i] + sz <= 128:
                resid_assign[br] = (BR + i, bins[i])
                bins[i] += sz
                placed = True
                break
        if not placed:
            bins.append(sz)
            resid_assign[br] = (BR + len(bins) - 1, 0)
    n_tiles = BR + len(bins)
    plan["n_tiles"] = n_tiles

    # copy pieces: band slot sub-ranges -> (tile, tile partition offset).
    # Copies are extended over the 32-alignment pad rows (their data is a
    # harmless finite duplicate; their W rows are zero), which leaves no
    # unwritten rows below each tile's used extent and costs nothing (engine
    # copy time depends only on the free-dim size).
    # Engine partition windows are buddy-aligned: from base b != 0 an access
    # must not cross the b + (b & -b) boundary; base 0 is unrestricted.
    used_rows = [0] * n_tiles
    for br in range(BR):
        for (vv, slot_lo, n) in plan["bands"][br]:
            lo = slot_lo
            remaining = ((n + 31) // 32) * 32
            while remaining > 0:
                chunk = lo // 128
                in_chunk = lo % 128
                take = min(remaining, 128 - in_chunk)
                if chunk == 0:
                    tid, tofs = br, in_chunk
                else:
                    tid, base = resid_assign[br]
                    tofs = base + in_chunk
                off = 0
                while off < take:
                    b = tofs + off
                    lim = take - off if b == 0 else min(take - off,
                                                        (b & -b))
                    plan["pieces"].append((br, vv, tid, tofs + off,
                                           lo + off, lim))
                    off += lim
                used_rows[tid] = max(used_rows[tid], tofs + take)
                lo += take
                remaining -= take
    plan["used_rows"] = used_rows

    # W rows per tile: tile partition row -> (br, original g) or None
    rows = [[None] * 128 for _ in range(n_tiles)]
    for br in range(BR):
        pos = 0
        for (vv, slot_lo, n) in plan["bands"][br]:
            for i in range(n):
                s = slot_lo + i
                g_orig = int(perms[br][pos + i])
                chunk = s // 128
                if chunk == 0:
                    tid, tofs = br, s
                else:
                    tid, base = resid_assign[br]
                    tofs = base + (s % 128)
                rows[tid][tofs] = (br, g_orig)
            pos += n
    plan["tile_rows"] = rows
    return plan


def _build_program(plan):
    nc = bacc.Bacc("TRN2", target_bir_lowering=False, debug=False,
                   num_devices=N_CORES)

    S = plan["S"]
    n_tiles = plan["n_tiles"]
    Smax = max(S)

    xs = [nc.dram_tensor(f"x{i}", [NK, 128, C], mybir.dt.float32,
                         kind="ExternalInput") for i in range(BR)]
    idxt_d = nc.dram_tensor("idxt", [128, BR * NK * (Smax // 16)],
                            mybir.dt.int16, kind="ExternalInput")
    wt_d = nc.dram_tensor("wt", [128, n_tiles * OC], mybir.dt.float32,
                          kind="ExternalInput")
    # half-major output layout: a conv-half DMA writes (b, q') as one
    # 784B contiguous run per o-row; host reassembles [B, OC, 14, 14]
    out_d = nc.dram_tensor("out", [2, OC, B, 98], mybir.dt.float32,
                           kind="ExternalOutput")

    f32 = mybir.dt.float32
    bf16 = mybir.dt.bfloat16
    KB = 4   # k-blocks per x DMA tile

    pieces_by_brk = {}
    for br, vv, tid, tofs, slot_lo, n in plan["pieces"]:
        pieces_by_brk.setdefault((br, slot_lo // 128), []).append(
            (vv, tid, tofs, slot_lo % 128, n))

    with tile.TileContext(nc) as tc, ExitStack() as ctx:
        cpool = ctx.enter_context(tc.tile_pool(name="const", bufs=1))
        xpool = ctx.enter_context(tc.tile_pool(name="xin", bufs=3))
        ppool = ctx.enter_context(tc.tile_pool(name="picked", bufs=6))
        featp = ctx.enter_context(tc.tile_pool(name="feats", bufs=1))
        opool = ctx.enter_context(tc.tile_pool(name="ostage", bufs=4))
        t2p = ctx.enter_context(tc.tile_pool(name="ps_t2", bufs=4, space="PSUM"))
        cvp = ctx.enter_context(tc.tile_pool(name="ps_cv", bufs=3, space="PSUM"))

        ident = cpool.tile([128, 128], f32)
        masks.make_identity(nc, ident[:])
        idxt = cpool.tile([128, BR * NK * (Smax // 16)], mybir.dt.int16)
        nc.sync.dma_start(idxt[:], idxt_d.ap())
        wt = cpool.tile([128, n_tiles * OC], f32)
        nc.sync.dma_start(wt[:], wt_d.ap())
        wtb = cpool.tile([128, n_tiles * OC], bf16)
        nc.vector.tensor_copy(wtb[:], wt[:])

        # feats split by q-half (98 cols each) so half-0 conv reads never
        # falsely depend on half-1 crop writes
        feats = [[featp.tile([128, B * 98], bf16, name=f"feat{h}_{i}")
                  for i in range(n_tiles)] for h in range(2)]

        rr = 0  # engine round-robin for copies

        def vcopy(dst, src):
            nonlocal rr
            if rr % 2 == 0:
                nc.vector.tensor_copy(dst, src)
            else:
                nc.scalar.copy(dst, src)
            rr += 1

        # only rows above each tile's written extent need zeroing (W rows
        # for pad slots are zero, so written pad rows are harmless)
        for h in range(2):
            for i in range(n_tiles):
                u = plan["used_rows"][i]
                while u < 128:
                    span = 128 - u if u == 0 else min(128 - u, u & -u)
                    nc.vector.memset(feats[h][i][u:u + span, :], 0.0)
                    u += span

        def conv_half(h):
            # matmul N=392 (4 batches); stage 8 batches per DMA so output
            # runs are 8*98*4 = 3136B and only 24 DMAs total are issued
            for grp in range(2):
                for oc in range(6):
                    ot = opool.tile([128, 2 * 392], f32)
                    for sub in range(2):
                        q4 = grp * 2 + sub
                        pc = cvp.tile([128, 392], f32)
                        for t in range(n_tiles):
                            lhsT = wtb[:, t * OC + oc * 128:
                                       t * OC + (oc + 1) * 128]
                            rhs = feats[h][t][:, q4 * 392:(q4 + 1) * 392]
                            nc.tensor.matmul(pc[:], lhsT, rhs,
                                             start=(t == 0),
                                             stop=(t == n_tiles - 1))
                        vcopy(ot[:, sub * 392:(sub + 1) * 392], pc[:])
                    dd = out_d.ap()[h, oc * 128:(oc + 1) * 128,
                                    grp * 8:(grp + 1) * 8, :]
                    nc.scalar.dma_start(dd, ot[:].rearrange(
                        "o (b q) -> o b q", b=8))

        # gather/crop in two q-half waves so half 0's conv overlaps half 1's
        # input streaming
        for half in range(2):
            kos = range(0, 4) if half == 0 else range(4, NK // KB)
            for br in range(BR):
                sb = S[br]
                xv = xs[br].ap().rearrange("(ko ki) pb c -> ko pb ki c", ki=KB)
                for ko in kos:
                    xt = xpool.tile([128, KB * C], f32)
                    nc.sync.dma_start(xt[:], xv[ko])
                    for ki in range(KB):
                        k = ko * KB + ki
                        pk = ppool.tile([128, Smax], f32)
                        icol = (br * NK + k) * (Smax // 16)
                        nc.gpsimd.ap_gather(
                            pk[:, :sb], xt[:, ki * C:(ki + 1) * C],
                            idxt[:, icol:icol + sb // 16],
                            channels=128, num_elems=C, d=1, num_idxs=sb)
                        for chunk in range((sb + 127) // 128):
                            cn = min(128, sb - 128 * chunk)
                            plist = pieces_by_brk.get((br, chunk))
                            if not plist:
                                continue
                            pt2 = t2p.tile([128, 128], f32)
                            nc.tensor.transpose(
                                pt2[:cn],
                                pk[:, 128 * chunk:128 * chunk + cn], ident[:])
                            # crop bands -> feats tiles, 32-aligned bases
                            for vv, tid, tofs, plo, n in plist:
                                geo = _band_geometry(k, vv // 2, vv % 2)
                                if geo is None:
                                    continue
                                pl_lo, pl_hi, q_base = geo
                                npl = pl_hi - pl_lo
                                qh = q_base // 98
                                ql = q_base - 98 * qh
                                src = pt2[plo:plo + n].rearrange(
                                    "g (pl b) -> g pl b",
                                    pl=8)[:, pl_lo:pl_hi, :]
                                dst = feats[qh][tid][tofs:tofs + n].rearrange(
                                    "g (b q) -> g q b",
                                    q=98)[:, ql:ql + npl, :]
                                vcopy(dst, src)
            conv_half(half)

    nc.compile()
    return nc


def _prep_aux(idx, offh, offw, W, plan):
    """Host-side index/layout preprocessing (relayout + address arithmetic)."""
    idx = np.asarray(idx)
    W = np.asarray(W, dtype=np.float32)
    perms = plan["perms"]
    Smax = max(plan["S"])

    # padded sorted index array per branch: [Smax, 256]
    idx_pad = np.zeros((BR, Smax, P), np.int64)
    for br in range(BR):
        pos = 0
        for (vv, slot_lo, n) in plan["bands"][br]:
            idx_pad[br, slot_lo:slot_lo + n] = idx[br][perms[br][pos:pos + n]]
            pos += n

    # gather index tiles: partition 16*j + r (j local position, r batch row),
    # free (br, k, s): value idx_pad[br, s*16+r, k*8+j]
    t = idx_pad.reshape(BR, Smax // 16, 16, NK, 8)   # [br, s, r, k, j]
    t = t.transpose(4, 2, 0, 3, 1)                   # [j, r, br, k, s]
    idxt = np.ascontiguousarray(
        t.reshape(128, BR * NK * (Smax // 16))).astype(np.int16)

    # W tiles: [g row, tile, o]; zero rows for pad slots
    Wr = W.reshape(OC, BR, 128)                      # [o, br, g]
    n_tiles = plan["n_tiles"]
    wt = np.zeros((128, n_tiles, OC), np.float32)
    for tid in range(n_tiles):
        for row in range(128):
            ent = plan["tile_rows"][tid][row]
            if ent is not None:
                br, g_orig = ent
                wt[row, tid] = Wr[:, br, g_orig]
    wt = np.ascontiguousarray(wt.reshape(128, n_tiles * OC))
    return idxt, wt


def _relayout_x(xc):
    """[16, 512, 256] -> [32, 128, 512]: out[k, pl*16+b, c] = x[b, c, 8k+pl]."""
    t = xc.reshape(B, C, NK, 8)                  # [b, c, k, pl]
    t = t.transpose(2, 3, 0, 1)                  # [k, pl, b, c]
    return np.ascontiguousarray(t.reshape(NK, 128, C))


def kernel(x1, x3, x5, W, idx, offh, offw):
    x1 = np.asarray(x1, dtype=np.float32)
    x3 = np.asarray(x3, dtype=np.float32)
    x5 = np.asarray(x5, dtype=np.float32)
    Bfull = x1.shape[0]
    assert Bfull == N_CORES * B

    offh = np.asarray(offh).astype(np.int64)
    offw = np.asarray(offw).astype(np.int64)
    plan = _plan(offh, offw)
    idxt, wt = _prep_aux(idx, offh, offw, W, plan)

    key = (tuple(plan["S"]), plan["n_tiles"],
           tuple(plan["pieces"]))
    if _CACHE.get("key") != key:
        _CACHE["nc"] = _build_program(plan)
        _CACHE["key"] = key
    nc = _CACHE["nc"]

    in_maps = []
    for core in range(N_CORES):
        sl = slice(core * B, (core + 1) * B)
        in_maps.append({
            "x0": _relayout_x(x1[sl].reshape(B, C, P)),
            "x1": _relayout_x(x3[sl].reshape(B, C, P)),
            "x2": _relayout_x(x5[sl].reshape(B, C, P)),
            "idxt": idxt,
            "wt": wt,
        })

    res = bass_utils.run_bass_kernel_spmd(nc, in_maps, list(range(N_CORES)))
    outs = []
    for i in range(N_CORES):
        oh = res.results[i]["out"]               # [2, OC, B, 98]
        o = oh.transpose(2, 1, 0, 3).reshape(B, OC, 14, 14)
        outs.append(o)
    return np.concatenate(outs, axis=0)

